# revision 5
# baseline (speedup 1.0000x reference)
"""8-core Trainium2 Bass kernel for the GNN message-passing classifier.

Strategy (self-contained; shapes hardcoded):
- Nodes padded 50000->50176, sharded 8 ways (6272/core = 49 blocks of 128).
- Edges assigned to the core owning dst, sorted by dst, packed into
  128-edge tiles per 128-node block, split by src half (int16 gather reach).
- Per level: dma_gather cur[src] rows (512B) from an AllGather'd node-major
  table in HBM; segmented reduce via PE matmuls with on-device one-hot
  selection matrices (is_equal vs iota); conv matmul in feature-major;
  PE transposes produce the next node-major table.
- Graph pooling via one-hot matmul per block + AllReduce; MLP head +
  log_softmax/loss/acc computed on-device, replicated; core 0's output used.
"""
import sys
import numpy as np

sys.path.insert(0, "/opt/trn_rl_repo")

import concourse.bass as bass
import concourse.mybir as mybir
import concourse.tile as tile
from concourse import bacc
from concourse.bass_utils import run_bass_kernel_spmd

N = 50000
NPAD = 50176
NCORES = 8
PC = NPAD // NCORES          # 6272
NBLK = PC // 128             # 49
F = 128
HID = 256
NCLS = 10
G = 128
SPLIT = 32768
MAX_LV = 3
CB = 2                       # blocks per gather chunk

FP32 = mybir.dt.float32
I16 = mybir.dt.int16
AF = mybir.ActivationFunctionType
ALU = mybir.AluOpType
AX = mybir.AxisListType

# ----------------------------------------------------------------- host prep

def _build_half(es_h, ed_h, T):
    TOT = NBLK * T * 128
    idx = np.zeros((NCORES, TOT), np.int16)
    dst = np.full((NCORES, TOT), -1.0, np.float32)
    if len(es_h):
        B = ed_h // 128
        first = np.searchsorted(B, np.arange(NCORES * NBLK), side="left")
        rank = np.arange(len(es_h)) - first[B]
        c = B // NBLK
        b = B % NBLK
        j = (b * T + rank // 128) * 128 + (rank % 128)
        idx[c, j] = es_h.astype(np.int16)
        dst[c, j] = (ed_h - (c * PC + b * 128)).astype(np.float32)
    return idx, dst


def _wrap16(arr):
    nc_, TOT = arr.shape
    w = np.zeros((nc_, 16, TOT // 16), np.int16)
    j = np.arange(TOT)
    w[:, j % 16, j // 16] = arr
    return np.tile(w, (1, 8, 1))


def _wrap128(arr):
    nc_, TOT = arr.shape
    w = np.zeros((nc_, 128, TOT // 128), arr.dtype)
    j = np.arange(TOT)
    w[:, j % 128, j // 128] = arr
    return w


def preprocess(node_feat, edge_src, edge_dst, graph_id):
    order = np.argsort(edge_dst, kind="stable")
    es = np.asarray(edge_src)[order].astype(np.int64)
    ed = np.asarray(edge_dst)[order].astype(np.int64)
    lo = es < SPLIT

    blk = ed // 128
    cnt_lo = np.bincount(blk[lo], minlength=NCORES * NBLK)
    cnt_hi = np.bincount(blk[~lo], minlength=NCORES * NBLK)
    T_lo = max(1, int(np.ceil(cnt_lo.max() / 128)))
    T_hi = max(1, int(np.ceil(cnt_hi.max() / 128)))

    idx_lo, dst_lo = _build_half(es[lo], ed[lo], T_lo)
    idx_hi, dst_hi = _build_half(es[~lo] - SPLIT, ed[~lo], T_hi)

    nf = np.zeros((NPAD, F), np.float32)
    nf[:N] = np.asarray(node_feat)
    nf_T = np.stack([
        np.ascontiguousarray(nf[c * PC:(c + 1) * PC].T) for c in range(NCORES)
    ])

    gid = np.full(NPAD, -1.0, np.float32)
    gid[:N] = np.asarray(graph_id).astype(np.float32)
    gid_nm = gid.reshape(NCORES, NBLK, 128).transpose(0, 2, 1).copy()

    return dict(
        T_lo=T_lo, T_hi=T_hi,
        idx_lo_w=_wrap16(idx_lo), idx_hi_w=_wrap16(idx_hi),
        dst_lo_w=_wrap128(dst_lo), dst_hi_w=_wrap128(dst_hi),
        nf_T=nf_T, gid_nm=gid_nm,
    )

# ------------------------------------------------------------- device build

def build(T_lo, T_hi, variant="full", max_lv=MAX_LV):
    nc = bacc.Bacc("TRN2", target_bir_lowering=False, debug=False)
    TOTL = NBLK * T_lo * 128
    TOTH = NBLK * T_hi * 128

    # inputs
    nf_T = nc.dram_tensor("nf_T", [F, PC], FP32, kind="ExternalInput")
    idx_lo = nc.dram_tensor("idx_lo", [128, TOTL // 16], I16, kind="ExternalInput")
    idx_hi = nc.dram_tensor("idx_hi", [128, TOTH // 16], I16, kind="ExternalInput")
    dst_lo = nc.dram_tensor("dst_lo", [128, NBLK * T_lo], FP32, kind="ExternalInput")
    dst_hi = nc.dram_tensor("dst_hi", [128, NBLK * T_hi], FP32, kind="ExternalInput")
    gid_d = nc.dram_tensor("gid", [128, NBLK], FP32, kind="ExternalInput")
    lblm_d = nc.dram_tensor("lblmask", [128, NCLS], FP32, kind="ExternalInput")
    iota_d = nc.dram_tensor("iota", [128, 128], FP32, kind="ExternalInput")
    ident_d = nc.dram_tensor("ident", [128, 128], FP32, kind="ExternalInput")
    wn2l_d = nc.dram_tensor("wn2lT", [F, F], FP32, kind="ExternalInput")
    wconv_d = nc.dram_tensor("wconvT", [F, F], FP32, kind="ExternalInput")
    wout_d = nc.dram_tensor("woutT", [F, F], FP32, kind="ExternalInput")
    wh1_d = nc.dram_tensor("wh1T", [F, HID], FP32, kind="ExternalInput")
    wh2_d = nc.dram_tensor("wh2T2", [128, 2, NCLS], FP32, kind="ExternalInput")
    bn2l_d = nc.dram_tensor("bn2l", [128, 1], FP32, kind="ExternalInput")
    bconv_d = nc.dram_tensor("bconv", [128, 1], FP32, kind="ExternalInput")
    bout_d = nc.dram_tensor("bout", [128, 1], FP32, kind="ExternalInput")
    bh1_d = nc.dram_tensor("bh1", [128, 2], FP32, kind="ExternalInput")
    bh2_d = nc.dram_tensor("bh2", [128, 1], FP32, kind="ExternalInput")
    mscale_d = nc.dram_tensor("mscale", [128, 1], FP32, kind="ExternalInput")
    ones_d = nc.dram_tensor("ones", [128, 1], FP32, kind="ExternalInput")

    # outputs
    logits_o = nc.dram_tensor("logits", [G, NCLS], FP32, kind="ExternalOutput")
    loss_o = nc.dram_tensor("loss", [1, 1], FP32, kind="ExternalOutput")
    acc_o = nc.dram_tensor("acc", [1, 1], FP32, kind="ExternalOutput")

    # internal dram
    slab_d = nc.dram_tensor("slab_d", [PC, F], FP32)
    tables = [
        nc.dram_tensor(f"table{l}", [NPAD, F], FP32, addr_space="Shared")
        for l in range(MAX_LV)
    ]
    ar_in = nc.dram_tensor("ar_in", [128, 128], FP32)
    ar_out = nc.dram_tensor("ar_out", [128, 128], FP32, addr_space="Shared")

    groups = [list(range(NCORES))]
    NG = [(g * 512, min(512, PC - g * 512)) for g in range((PC + 511) // 512)]
    chunks = [(b0, min(CB, NBLK - b0)) for b0 in range(0, NBLK, CB)]

    with tile.TileContext(nc) as tc:
        with (
            tc.tile_pool(name="res", bufs=1) as res,
            tc.tile_pool(name="big", bufs=1) as big,
            tc.tile_pool(name="glo", bufs=2) as glo_p,
            tc.tile_pool(name="ghi", bufs=2) as ghi_p,
            tc.tile_pool(name="selp", bufs=2) as selp,
            tc.tile_pool(name="ps_agg", bufs=2, space="PSUM") as ps_agg,
            tc.tile_pool(name="ps_conv", bufs=2, space="PSUM") as ps_conv,
            tc.tile_pool(name="ps_t", bufs=2, space="PSUM") as ps_t,
        ):
            def load(dram, shape, dtype=FP32, pool=res):
                t = pool.tile(shape, dtype, tag=f"ld_{dram.name}")
                nc.sync.dma_start(t[:], dram[:])
                return t

            idxl_sb = load(idx_lo, [128, TOTL // 16], I16)
            idxh_sb = load(idx_hi, [128, TOTH // 16], I16)
            dstl_sb = load(dst_lo, [128, NBLK * T_lo])
            dsth_sb = load(dst_hi, [128, NBLK * T_hi])
            gid_sb = load(gid_d, [128, NBLK])
            lblm_sb = load(lblm_d, [128, NCLS])
            iota_sb = load(iota_d, [128, 128])
            id_sb = load(ident_d, [128, 128])
            wn2l_sb = load(wn2l_d, [F, F])
            wconv_sb = load(wconv_d, [F, F])
            wout_sb = load(wout_d, [F, F])
            wh1_sb = load(wh1_d, [F, HID])
            wh2_sb = load(wh2_d, [128, 2, NCLS])
            bn2l_sb = load(bn2l_d, [128, 1])
            bconv_sb = load(bconv_d, [128, 1])
            bout_sb = load(bout_d, [128, 1])
            bh1_sb = load(bh1_d, [128, 2])
            bh2_sb = load(bh2_d, [128, 1])
            msc_sb = load(mscale_d, [128, 1])
            ones_sb = load(ones_d, [128, 1])

            im_T = res.tile([F, PC], FP32)    # input message, feature-major
            cur_T = res.tile([F, PC], FP32)
            agg_T = res.tile([F, PC], FP32)

            # ---- stage 0: im_T = wn2l.T^T @ nf_T + b ; cur_T = relu(im_T)
            nfs = big.tile([F, PC], FP32, tag="nf_stage")
            nc.sync.dma_start(nfs[:], nf_T[:])
            for g0, gn in NG:
                p = ps_conv.tile([128, 512], FP32, tag="pconv")
                nc.tensor.matmul(p[:, :gn], lhsT=wn2l_sb[:], rhs=nfs[:, g0:g0 + gn],
                                 start=True, stop=True)
                nc.scalar.activation(im_T[:, g0:g0 + gn], p[:, :gn], AF.Identity,
                                     bias=bn2l_sb[:])
                nc.scalar.activation(cur_T[:, g0:g0 + gn], p[:, :gn], AF.Relu,
                                     bias=bn2l_sb[:])

            # ---- 3 message-passing levels
            for lv in range(max_lv):
                # write cur_T -> node-major slab -> allgather -> table
                stag = big.tile([128, NBLK, F], FP32, tag="nf_stage")
                for b in range(NBLK):
                    pt = ps_t.tile([128, 128], FP32, tag="pt")
                    nc.tensor.transpose(pt[:], cur_T[:, b * 128:(b + 1) * 128], id_sb[:])
                    nc.scalar.activation(stag[:, b, :], pt[:], AF.Copy)
                nc.sync.dma_start(
                    slab_d.rearrange("(b p) f -> p b f", p=128), stag[:])
                nc.gpsimd.collective_compute(
                    "AllGather", ALU.bypass, replica_groups=groups,
                    ins=[slab_d[:]], outs=[tables[lv][:]])

                # gather + segmented reduce into agg_T
                if variant == "nogather":
                    nc.vector.tensor_copy(agg_T[:], cur_T[:])
                for b0, nb in (chunks if variant != "nogather" else []):
                    gl = glo_p.tile([128, CB * T_lo, F], FP32, tag="gl")
                    nc.gpsimd.dma_gather(
                        out_ap=gl[:, :nb * T_lo, :],
                        in_ap=tables[lv][0:SPLIT, :],
                        idxs_ap=idxl_sb[:, b0 * T_lo * 8:(b0 + nb) * T_lo * 8],
                        num_idxs=nb * T_lo * 128,
                        num_idxs_reg=nb * T_lo * 128,
                        elem_size=F, single_packet=False)
                    gh = ghi_p.tile([128, CB * T_hi, F], FP32, tag="gh")
                    nc.gpsimd.dma_gather(
                        out_ap=gh[:, :nb * T_hi, :],
                        in_ap=tables[lv][SPLIT:NPAD, :],
                        idxs_ap=idxh_sb[:, b0 * T_hi * 8:(b0 + nb) * T_hi * 8],
                        num_idxs=nb * T_hi * 128,
                        num_idxs_reg=nb * T_hi * 128,
                        elem_size=F, single_packet=False)
                    for bi in range(nb):
                        b = b0 + bi
                        sl = selp.tile([128, T_lo, 128], FP32, tag="sel_lo")
                        nc.vector.tensor_tensor(
                            out=sl[:],
                            in0=dstl_sb[:, b * T_lo:(b + 1) * T_lo, None]
                                .to_broadcast([128, T_lo, 128]),
                            in1=iota_sb[:, None, :].to_broadcast([128, T_lo, 128]),
                            op=ALU.is_equal)
                        sh = selp.tile([128, T_hi, 128], FP32, tag="sel_hi")
                        nc.vector.tensor_tensor(
                            out=sh[:],
                            in0=dsth_sb[:, b * T_hi:(b + 1) * T_hi, None]
                                .to_broadcast([128, T_hi, 128]),
                            in1=iota_sb[:, None, :].to_broadcast([128, T_hi, 128]),
                            op=ALU.is_equal)
                        pagg = ps_agg.tile([128, 128], FP32, tag="pagg")
                        for t in range(T_lo):
                            nc.tensor.matmul(
                                pagg[:], lhsT=gl[:, bi * T_lo + t, :],
                                rhs=sl[:, t, :], start=(t == 0), stop=False)
                        for t in range(T_hi):
                            nc.tensor.matmul(
                                pagg[:], lhsT=gh[:, bi * T_hi + t, :],
                                rhs=sh[:, t, :], start=False,
                                stop=(t == T_hi - 1))
                        nc.scalar.activation(
                            agg_T[:, b * 128:(b + 1) * 128], pagg[:], AF.Copy)

                # conv: cur_T = relu(wconv.T^T @ agg_T + im_T + b_conv)
                for g0, gn in NG:
                    p = ps_conv.tile([128, 512], FP32, tag="pconv")
                    nc.tensor.matmul(p[:, :gn], lhsT=wconv_sb[:],
                                     rhs=agg_T[:, g0:g0 + gn],
                                     start=True, stop=True)
                    nc.vector.tensor_tensor(out=p[:, :gn], in0=p[:, :gn],
                                            in1=im_T[:, g0:g0 + gn], op=ALU.add)
                    nc.scalar.activation(cur_T[:, g0:g0 + gn], p[:, :gn],
                                         AF.Relu, bias=bconv_sb[:])

            # ---- out stage: out_T = relu(wout.T^T @ cur_T + b_out) into agg_T
            for g0, gn in NG:
                p = ps_conv.tile([128, 512], FP32, tag="pconv")
                nc.tensor.matmul(p[:, :gn], lhsT=wout_sb[:],
                                 rhs=cur_T[:, g0:g0 + gn], start=True, stop=True)
                nc.scalar.activation(agg_T[:, g0:g0 + gn], p[:, :gn],
                                     AF.Relu, bias=bout_sb[:])

            # ---- pooling: embed partial [fo, g] = sum_b out_nm[b]^T-style matmul
            ppool = ps_agg.tile([128, 128], FP32, tag="pagg")
            for b in range(NBLK):
                pt = ps_t.tile([128, 128], FP32, tag="pt")
                nc.tensor.transpose(pt[:], agg_T[:, b * 128:(b + 1) * 128], id_sb[:])
                onm = selp.tile([128, 128], FP32, tag="sel_lo")
                nc.scalar.activation(onm[:], pt[:], AF.Copy)
                mt = selp.tile([128, 128], FP32, tag="sel_hi")
                nc.vector.tensor_tensor(
                    out=mt[:],
                    in0=gid_sb[:, b:b + 1].to_broadcast([128, 128]),
                    in1=iota_sb[:],
                    op=ALU.is_equal)
                nc.tensor.matmul(ppool[:], lhsT=onm[:], rhs=mt[:],
                                 start=(b == 0), stop=(b == NBLK - 1))
            poolp = res.tile([128, 128], FP32)
            nc.scalar.activation(poolp[:], ppool[:], AF.Copy)
            nc.sync.dma_start(ar_in[:], poolp[:])
            nc.gpsimd.collective_compute(
                "AllReduce", ALU.add, replica_groups=groups,
                ins=[ar_in[:]], outs=[ar_out[:]])
            emb_T = res.tile([128, 128], FP32)
            nc.sync.dma_start(emb_T[:], ar_out[:])
            nc.scalar.activation(emb_T[:], emb_T[:], AF.Relu)

            # ---- head
            h1a = res.tile([128, 128], FP32)
            h1b = res.tile([128, 128], FP32)
            p = ps_conv.tile([128, 512], FP32, tag="pconv")
            nc.tensor.matmul(p[:, :128], lhsT=wh1_sb[:, 0:128], rhs=emb_T[:],
                             start=True, stop=True)
            nc.scalar.activation(h1a[:], p[:, :128], AF.Relu, bias=bh1_sb[:, 0:1])
            p2 = ps_conv.tile([128, 512], FP32, tag="pconv")
            nc.tensor.matmul(p2[:, :128], lhsT=wh1_sb[:, 128:256], rhs=emb_T[:],
                             start=True, stop=True)
            nc.scalar.activation(h1b[:], p2[:, :128], AF.Relu, bias=bh1_sb[:, 1:2])

            plog = ps_agg.tile([128, 128], FP32, tag="pagg")
            nc.tensor.matmul(plog[:NCLS, :], lhsT=wh2_sb[:, 0, :], rhs=h1a[:],
                             start=True, stop=False)
            nc.tensor.matmul(plog[:NCLS, :], lhsT=wh2_sb[:, 1, :], rhs=h1b[:],
                             start=False, stop=True)
            rawT = res.tile([128, 128], FP32)
            nc.vector.memset(rawT[:], 0.0)
            nc.scalar.activation(rawT[:NCLS, :], plog[:NCLS, :], AF.Identity,
                                 bias=bh2_sb[:NCLS, :])
            pt2 = ps_t.tile([128, 128], FP32, tag="pt")
            nc.tensor.transpose(pt2[:], rawT[:], id_sb[:])
            raw_nm = res.tile([128, NCLS], FP32)
            nc.scalar.activation(raw_nm[:], pt2[:, :NCLS], AF.Copy)

            # log_softmax over free axis (10)
            mx = res.tile([128, 1], FP32)
            nc.vector.tensor_reduce(mx[:], raw_nm[:], axis=AX.X, op=ALU.max)
            xs = res.tile([128, NCLS], FP32)
            nc.vector.tensor_tensor(out=xs[:], in0=raw_nm[:],
                                    in1=mx[:].to_broadcast([128, NCLS]),
                                    op=ALU.subtract)
            ex = res.tile([128, NCLS], FP32)
            nc.scalar.activation(ex[:], xs[:], AF.Exp)
            ssum = res.tile([128, 1], FP32)
            nc.vector.tensor_reduce(ssum[:], ex[:], axis=AX.X, op=ALU.add)
            ls = res.tile([128, 1], FP32)
            nc.scalar.activation(ls[:], ssum[:], AF.Ln)
            logout = res.tile([128, NCLS], FP32)
            nc.vector.tensor_tensor(out=logout[:], in0=xs[:],
                                    in1=ls[:].to_broadcast([128, NCLS]),
                                    op=ALU.subtract)
            nc.sync.dma_start(logits_o[:], logout[:])

            # loss / acc
            tmp10 = res.tile([128, NCLS], FP32)
            nc.vector.tensor_tensor(out=tmp10[:], in0=raw_nm[:], in1=lblm_sb[:],
                                    op=ALU.mult)
            pick_raw = res.tile([128, 1], FP32)
            nc.vector.tensor_reduce(pick_raw[:], tmp10[:], axis=AX.X, op=ALU.add)
            stack2 = res.tile([128, 2], FP32)
            nc.vector.tensor_tensor(out=stack2[:, 1:2], in0=pick_raw[:],
                                    in1=mx[:], op=ALU.is_equal)
            tmp10b = res.tile([128, NCLS], FP32)
            nc.vector.tensor_tensor(out=tmp10b[:], in0=logout[:], in1=lblm_sb[:],
                                    op=ALU.mult)
            nc.vector.tensor_reduce(stack2[:, 0:1], tmp10b[:], axis=AX.X,
                                    op=ALU.add)
            ps_s = ps_t.tile([128, 128], FP32, tag="pt")
            nc.tensor.matmul(ps_s[:2, :1], lhsT=stack2[:], rhs=ones_sb[:],
                             start=True, stop=True)
            fin = res.tile([128, 1], FP32)
            nc.vector.tensor_tensor(out=fin[:2, :], in0=ps_s[:2, :1],
                                    in1=msc_sb[:2, :], op=ALU.mult)
            nc.sync.dma_start(loss_o[:], fin[0:1, :])
            nc.sync.dma_start(acc_o[:], fin[1:2, :])

    nc.compile()
    return nc


_CACHE = {}


def _get_nc(T_lo, T_hi):
    import os
    variant = os.environ.get("KVAR", "full")
    max_lv = int(os.environ.get("KLV", str(MAX_LV)))
    key = (T_lo, T_hi, variant, max_lv)
    if key not in _CACHE:
        _CACHE[key] = build(T_lo, T_hi, variant, max_lv)
    return _CACHE[key]


def run(inputs, trace=False):
    pre = preprocess(inputs["node_feat"], inputs["edge_src"],
                     inputs["edge_dst"], inputs["graph_id"])
    T_lo, T_hi = pre["T_lo"], pre["T_hi"]
    nc = _get_nc(T_lo, T_hi)

    labels = np.asarray(inputs["labels"])
    lblmask = np.zeros((G, NCLS), np.float32)
    lblmask[np.arange(G), labels] = 1.0
    iota = np.tile(np.arange(128, dtype=np.float32), (128, 1))
    ident = np.eye(128, dtype=np.float32)
    mscale = np.zeros((128, 1), np.float32)
    mscale[0, 0] = -1.0 / G
    mscale[1, 0] = 1.0 / G
    ones = np.ones((128, 1), np.float32)
    b128 = lambda v: np.asarray(v, np.float32).reshape(128, 1)
    bh1 = np.asarray(inputs["b_h1"], np.float32).reshape(2, 128).T.copy()
    bh2 = np.zeros((128, 1), np.float32)
    bh2[:NCLS, 0] = np.asarray(inputs["b_h2"], np.float32)
    wh2T = np.asarray(inputs["w_h2"], np.float32).T  # [256, 10]
    wh2T2 = np.stack([wh2T[:128], wh2T[128:]], axis=1).copy()  # [128, 2, 10]

    shared = dict(
        lblmask=lblmask, iota=iota, ident=ident, mscale=mscale, ones=ones,
        wn2lT=np.ascontiguousarray(np.asarray(inputs["w_n2l"], np.float32).T),
        wconvT=np.ascontiguousarray(np.asarray(inputs["w_conv"], np.float32).T),
        woutT=np.ascontiguousarray(np.asarray(inputs["w_out"], np.float32).T),
        wh1T=np.ascontiguousarray(np.asarray(inputs["w_h1"], np.float32).T),
        wh2T2=wh2T2,
        bn2l=b128(inputs["b_n2l"]), bconv=b128(inputs["b_conv"]),
        bout=b128(inputs["b_out"]), bh1=bh1, bh2=bh2,
    )
    in_maps = []
    for c in range(NCORES):
        m = dict(shared)
        m["nf_T"] = pre["nf_T"][c]
        m["idx_lo"] = pre["idx_lo_w"][c]
        m["idx_hi"] = pre["idx_hi_w"][c]
        m["dst_lo"] = pre["dst_lo_w"][c]
        m["dst_hi"] = pre["dst_hi_w"][c]
        m["gid"] = pre["gid_nm"][c]
        in_maps.append(m)

    res = run_bass_kernel_spmd(nc, in_maps, core_ids=list(range(NCORES)),
                               trace=trace)
    r0 = res.results[0]
    logits = r0["logits"].astype(np.float32)
    loss = np.float32(r0["loss"].reshape(())[()])
    acc = np.float32(r0["acc"].reshape(())[()])
    return (logits, loss, acc), res


def kernel(**inputs):
    (logits, loss, acc), _ = run(inputs, trace=False)
    return logits, loss, acc


# revision 7
# speedup vs baseline: 1.5964x; 1.5964x over previous
"""8-core Trainium2 Bass kernel for the GNN message-passing classifier.

Strategy (self-contained; shapes hardcoded):
- Nodes padded 50000->50176, sharded 8 ways (6272/core = 49 blocks of 128).
- Edges assigned to the core owning dst, sorted by dst, packed into
  128-edge tiles per 128-node block, split by src half (int16 gather reach).
- Per level: dma_gather cur[src] rows (512B) from an AllGather'd node-major
  table in HBM; segmented reduce via PE matmuls with on-device one-hot
  selection matrices (is_equal vs iota); conv matmul in feature-major;
  PE transposes produce the next node-major table.
- Graph pooling via one-hot matmul per block + AllReduce; MLP head +
  log_softmax/loss/acc computed on-device, replicated; core 0's output used.
"""
import sys
import numpy as np

sys.path.insert(0, "/opt/trn_rl_repo")

import concourse.bass as bass
import concourse.mybir as mybir
import concourse.tile as tile
from concourse import bacc
from concourse.bass_utils import run_bass_kernel_spmd

N = 50000
NPAD = 50176
NCORES = 8
PC = NPAD // NCORES          # 6272
NBLK = PC // 128             # 49
F = 128
HID = 256
NCLS = 10
G = 128
SPLIT = 32768
MAX_LV = 3
CB = 2                       # blocks per gather chunk

FP32 = mybir.dt.float32
I16 = mybir.dt.int16
AF = mybir.ActivationFunctionType
ALU = mybir.AluOpType
AX = mybir.AxisListType

# ----------------------------------------------------------------- host prep

def _build_half(es_h, ed_h, T):
    TOT = NBLK * T * 128
    idx = np.zeros((NCORES, TOT), np.int16)
    dst = np.full((NCORES, TOT), -1.0, np.float32)
    if len(es_h):
        B = ed_h // 128
        first = np.searchsorted(B, np.arange(NCORES * NBLK), side="left")
        rank = np.arange(len(es_h)) - first[B]
        c = B // NBLK
        b = B % NBLK
        j = (b * T + rank // 128) * 128 + (rank % 128)
        idx[c, j] = es_h.astype(np.int16)
        dst[c, j] = (ed_h - (c * PC + b * 128)).astype(np.float32)
    return idx, dst


def _wrap16(arr):
    nc_, TOT = arr.shape
    w = np.zeros((nc_, 16, TOT // 16), np.int16)
    j = np.arange(TOT)
    w[:, j % 16, j // 16] = arr
    return np.tile(w, (1, 8, 1))


def _wrap128(arr):
    nc_, TOT = arr.shape
    w = np.zeros((nc_, 128, TOT // 128), arr.dtype)
    j = np.arange(TOT)
    w[:, j % 128, j // 128] = arr
    return w


def preprocess(node_feat, edge_src, edge_dst, graph_id):
    order = np.argsort(edge_dst, kind="stable")
    es = np.asarray(edge_src)[order].astype(np.int64)
    ed = np.asarray(edge_dst)[order].astype(np.int64)
    lo = es < SPLIT

    blk = ed // 128
    cnt_lo = np.bincount(blk[lo], minlength=NCORES * NBLK)
    cnt_hi = np.bincount(blk[~lo], minlength=NCORES * NBLK)
    T_lo = -4 * (-int(np.ceil(cnt_lo.max() / 128)) // 4)
    T_hi = -4 * (-int(np.ceil(cnt_hi.max() / 128)) // 4)

    idx_lo, dst_lo = _build_half(es[lo], ed[lo], T_lo)
    idx_hi, dst_hi = _build_half(es[~lo] - SPLIT, ed[~lo], T_hi)

    nf = np.zeros((NPAD, F), np.float32)
    nf[:N] = np.asarray(node_feat)
    nf_T = np.stack([
        np.ascontiguousarray(nf[c * PC:(c + 1) * PC].T) for c in range(NCORES)
    ])

    gid = np.full(NPAD, -1.0, np.float32)
    gid[:N] = np.asarray(graph_id).astype(np.float32)
    gid_nm = gid.reshape(NCORES, NBLK, 128).transpose(0, 2, 1).copy()

    return dict(
        T_lo=T_lo, T_hi=T_hi,
        idx_lo_w=_wrap16(idx_lo), idx_hi_w=_wrap16(idx_hi),
        dst_lo_w=_wrap128(dst_lo), dst_hi_w=_wrap128(dst_hi),
        nf_T=nf_T, gid_nm=gid_nm,
    )

# ------------------------------------------------------------- device build

def build(T_lo, T_hi, variant="full", max_lv=MAX_LV):
    nc = bacc.Bacc("TRN2", target_bir_lowering=False, debug=False,
                   num_swdge_queues=4)
    TOTL = NBLK * T_lo * 128
    TOTH = NBLK * T_hi * 128

    # inputs
    nf_T = nc.dram_tensor("nf_T", [F, PC], FP32, kind="ExternalInput")
    idx_lo = nc.dram_tensor("idx_lo", [128, TOTL // 16], I16, kind="ExternalInput")
    idx_hi = nc.dram_tensor("idx_hi", [128, TOTH // 16], I16, kind="ExternalInput")
    dst_lo = nc.dram_tensor("dst_lo", [128, NBLK * T_lo], FP32, kind="ExternalInput")
    dst_hi = nc.dram_tensor("dst_hi", [128, NBLK * T_hi], FP32, kind="ExternalInput")
    gid_d = nc.dram_tensor("gid", [128, NBLK], FP32, kind="ExternalInput")
    lblm_d = nc.dram_tensor("lblmask", [128, NCLS], FP32, kind="ExternalInput")
    iota_d = nc.dram_tensor("iota", [128, 128], FP32, kind="ExternalInput")
    ident_d = nc.dram_tensor("ident", [128, 128], FP32, kind="ExternalInput")
    wn2l_d = nc.dram_tensor("wn2lT", [F, F], FP32, kind="ExternalInput")
    wconv_d = nc.dram_tensor("wconvT", [F, F], FP32, kind="ExternalInput")
    wout_d = nc.dram_tensor("woutT", [F, F], FP32, kind="ExternalInput")
    wh1_d = nc.dram_tensor("wh1T", [F, HID], FP32, kind="ExternalInput")
    wh2_d = nc.dram_tensor("wh2T2", [128, 2, NCLS], FP32, kind="ExternalInput")
    bn2l_d = nc.dram_tensor("bn2l", [128, 1], FP32, kind="ExternalInput")
    bconv_d = nc.dram_tensor("bconv", [128, 1], FP32, kind="ExternalInput")
    bout_d = nc.dram_tensor("bout", [128, 1], FP32, kind="ExternalInput")
    bh1_d = nc.dram_tensor("bh1", [128, 2], FP32, kind="ExternalInput")
    bh2_d = nc.dram_tensor("bh2", [128, 1], FP32, kind="ExternalInput")
    mscale_d = nc.dram_tensor("mscale", [128, 1], FP32, kind="ExternalInput")
    ones_d = nc.dram_tensor("ones", [128, 1], FP32, kind="ExternalInput")

    # outputs
    logits_o = nc.dram_tensor("logits", [G, NCLS], FP32, kind="ExternalOutput")
    loss_o = nc.dram_tensor("loss", [1, 1], FP32, kind="ExternalOutput")
    acc_o = nc.dram_tensor("acc", [1, 1], FP32, kind="ExternalOutput")

    # internal dram
    slab_d = nc.dram_tensor("slab_d", [PC, F], FP32)
    tables = [
        nc.dram_tensor(f"table{l}", [NPAD, F], FP32, addr_space="Shared")
        for l in range(MAX_LV)
    ]
    ar_in = nc.dram_tensor("ar_in", [128, 128], FP32)
    ar_out = nc.dram_tensor("ar_out", [128, 128], FP32, addr_space="Shared")

    groups = [list(range(NCORES))]
    NG = [(g * 512, min(512, PC - g * 512)) for g in range((PC + 511) // 512)]
    chunks = [(b0, min(CB, NBLK - b0)) for b0 in range(0, NBLK, CB)]

    with tile.TileContext(nc) as tc:
        with (
            tc.tile_pool(name="res", bufs=1) as res,
            tc.tile_pool(name="big", bufs=1) as big,
            tc.tile_pool(name="glo", bufs=2) as glo_p,
            tc.tile_pool(name="ghi", bufs=2) as ghi_p,
            tc.tile_pool(name="selp", bufs=2) as selp,
            tc.tile_pool(name="ps_agg", bufs=2, space="PSUM") as ps_agg,
            tc.tile_pool(name="ps_conv", bufs=2, space="PSUM") as ps_conv,
            tc.tile_pool(name="ps_t", bufs=2, space="PSUM") as ps_t,
        ):
            def load(dram, shape, dtype=FP32, pool=res):
                t = pool.tile(shape, dtype, tag=f"ld_{dram.name}")
                nc.sync.dma_start(t[:], dram[:])
                return t

            idxl_sb = load(idx_lo, [128, TOTL // 16], I16)
            idxh_sb = load(idx_hi, [128, TOTH // 16], I16)
            dstl_sb = load(dst_lo, [128, NBLK * T_lo])
            dsth_sb = load(dst_hi, [128, NBLK * T_hi])
            gid_sb = load(gid_d, [128, NBLK])
            lblm_sb = load(lblm_d, [128, NCLS])
            iota_sb = load(iota_d, [128, 128])
            id_sb = load(ident_d, [128, 128])
            wn2l_sb = load(wn2l_d, [F, F])
            wconv_sb = load(wconv_d, [F, F])
            wout_sb = load(wout_d, [F, F])
            wh1_sb = load(wh1_d, [F, HID])
            wh2_sb = load(wh2_d, [128, 2, NCLS])
            bn2l_sb = load(bn2l_d, [128, 1])
            bconv_sb = load(bconv_d, [128, 1])
            bout_sb = load(bout_d, [128, 1])
            bh1_sb = load(bh1_d, [128, 2])
            bh2_sb = load(bh2_d, [128, 1])
            msc_sb = load(mscale_d, [128, 1])
            ones_sb = load(ones_d, [128, 1])

            im_T = res.tile([F, PC], FP32)    # input message, feature-major
            cur_T = res.tile([F, PC], FP32)
            agg_T = res.tile([F, PC], FP32)

            # ---- stage 0: im_T = wn2l.T^T @ nf_T + b ; cur_T = relu(im_T)
            nfs = big.tile([F, PC], FP32, tag="nf_stage")
            nc.sync.dma_start(nfs[:], nf_T[:])
            for g0, gn in NG:
                p = ps_conv.tile([128, 512], FP32, tag="pconv")
                nc.tensor.matmul(p[:, :gn], lhsT=wn2l_sb[:], rhs=nfs[:, g0:g0 + gn],
                                 start=True, stop=True)
                nc.scalar.activation(im_T[:, g0:g0 + gn], p[:, :gn], AF.Identity,
                                     bias=bn2l_sb[:])
                nc.scalar.activation(cur_T[:, g0:g0 + gn], p[:, :gn], AF.Relu,
                                     bias=bn2l_sb[:])

            # ---- 3 message-passing levels
            qctr = [0]
            def nextq():
                qctr[0] += 1
                return qctr[0] % 4
            for lv in range(max_lv):
                # write cur_T -> node-major slab -> allgather -> table
                stag = big.tile([128, NBLK, F], FP32, tag="nf_stage")
                for b in range(NBLK):
                    pt = ps_t.tile([128, 128], FP32, tag="pt")
                    nc.tensor.transpose(pt[:], cur_T[:, b * 128:(b + 1) * 128], id_sb[:])
                    nc.scalar.activation(stag[:, b, :], pt[:], AF.Copy)
                nc.sync.dma_start(
                    slab_d.rearrange("(b p) f -> p b f", p=128), stag[:])
                nc.gpsimd.collective_compute(
                    "AllGather", ALU.bypass, replica_groups=groups,
                    ins=[slab_d[:]], outs=[tables[lv][:]])

                # gather + segmented reduce into agg_T
                if variant == "nogather":
                    nc.vector.tensor_copy(agg_T[:], cur_T[:])
                for b0, nb in (chunks if variant != "nogather" else []):
                    gl = glo_p.tile([128, CB * T_lo, F], FP32, tag="gl")
                    for gi in range(nb * T_lo * 128 // 512):
                        c0 = b0 * T_lo * 8 + gi * 32
                        nc.gpsimd.dma_gather(
                            out_ap=gl[:, gi * 4:(gi + 1) * 4, :],
                            in_ap=tables[lv][0:SPLIT, :],
                            idxs_ap=idxl_sb[:, c0:c0 + 32],
                            num_idxs=512, num_idxs_reg=512,
                            elem_size=F, single_packet=True,
                            queue_num=nextq())
                    gh = ghi_p.tile([128, CB * T_hi, F], FP32, tag="gh")
                    for gi in range(nb * T_hi * 128 // 512):
                        c0 = b0 * T_hi * 8 + gi * 32
                        nc.gpsimd.dma_gather(
                            out_ap=gh[:, gi * 4:(gi + 1) * 4, :],
                            in_ap=tables[lv][SPLIT:NPAD, :],
                            idxs_ap=idxh_sb[:, c0:c0 + 32],
                            num_idxs=512, num_idxs_reg=512,
                            elem_size=F, single_packet=True,
                            queue_num=nextq())
                    for bi in range(nb):
                        b = b0 + bi
                        sl = selp.tile([128, T_lo, 128], FP32, tag="sel_lo")
                        nc.vector.tensor_tensor(
                            out=sl[:],
                            in0=dstl_sb[:, b * T_lo:(b + 1) * T_lo, None]
                                .to_broadcast([128, T_lo, 128]),
                            in1=iota_sb[:, None, :].to_broadcast([128, T_lo, 128]),
                            op=ALU.is_equal)
                        sh = selp.tile([128, T_hi, 128], FP32, tag="sel_hi")
                        nc.vector.tensor_tensor(
                            out=sh[:],
                            in0=dsth_sb[:, b * T_hi:(b + 1) * T_hi, None]
                                .to_broadcast([128, T_hi, 128]),
                            in1=iota_sb[:, None, :].to_broadcast([128, T_hi, 128]),
                            op=ALU.is_equal)
                        pagg = ps_agg.tile([128, 128], FP32, tag="pagg")
                        for t in range(T_lo):
                            nc.tensor.matmul(
                                pagg[:], lhsT=gl[:, bi * T_lo + t, :],
                                rhs=sl[:, t, :], start=(t == 0), stop=False)
                        for t in range(T_hi):
                            nc.tensor.matmul(
                                pagg[:], lhsT=gh[:, bi * T_hi + t, :],
                                rhs=sh[:, t, :], start=False,
                                stop=(t == T_hi - 1))
                        nc.scalar.activation(
                            agg_T[:, b * 128:(b + 1) * 128], pagg[:], AF.Copy)

                # conv: cur_T = relu(wconv.T^T @ agg_T + im_T + b_conv)
                for g0, gn in NG:
                    p = ps_conv.tile([128, 512], FP32, tag="pconv")
                    nc.tensor.matmul(p[:, :gn], lhsT=wconv_sb[:],
                                     rhs=agg_T[:, g0:g0 + gn],
                                     start=True, stop=True)
                    nc.vector.tensor_tensor(out=p[:, :gn], in0=p[:, :gn],
                                            in1=im_T[:, g0:g0 + gn], op=ALU.add)
                    nc.scalar.activation(cur_T[:, g0:g0 + gn], p[:, :gn],
                                         AF.Relu, bias=bconv_sb[:])

            # ---- out stage: out_T = relu(wout.T^T @ cur_T + b_out) into agg_T
            for g0, gn in NG:
                p = ps_conv.tile([128, 512], FP32, tag="pconv")
                nc.tensor.matmul(p[:, :gn], lhsT=wout_sb[:],
                                 rhs=cur_T[:, g0:g0 + gn], start=True, stop=True)
                nc.scalar.activation(agg_T[:, g0:g0 + gn], p[:, :gn],
                                     AF.Relu, bias=bout_sb[:])

            # ---- pooling: embed partial [fo, g] = sum_b out_nm[b]^T-style matmul
            ppool = ps_agg.tile([128, 128], FP32, tag="pagg")
            for b in range(NBLK):
                pt = ps_t.tile([128, 128], FP32, tag="pt")
                nc.tensor.transpose(pt[:], agg_T[:, b * 128:(b + 1) * 128], id_sb[:])
                onm = selp.tile([128, 128], FP32, tag="sel_lo")
                nc.scalar.activation(onm[:], pt[:], AF.Copy)
                mt = selp.tile([128, 128], FP32, tag="sel_hi")
                nc.vector.tensor_tensor(
                    out=mt[:],
                    in0=gid_sb[:, b:b + 1].to_broadcast([128, 128]),
                    in1=iota_sb[:],
                    op=ALU.is_equal)
                nc.tensor.matmul(ppool[:], lhsT=onm[:], rhs=mt[:],
                                 start=(b == 0), stop=(b == NBLK - 1))
            poolp = res.tile([128, 128], FP32)
            nc.scalar.activation(poolp[:], ppool[:], AF.Copy)
            nc.sync.dma_start(ar_in[:], poolp[:])
            nc.gpsimd.collective_compute(
                "AllReduce", ALU.add, replica_groups=groups,
                ins=[ar_in[:]], outs=[ar_out[:]])
            emb_T = res.tile([128, 128], FP32)
            nc.sync.dma_start(emb_T[:], ar_out[:])
            nc.scalar.activation(emb_T[:], emb_T[:], AF.Relu)

            # ---- head
            h1a = res.tile([128, 128], FP32)
            h1b = res.tile([128, 128], FP32)
            p = ps_conv.tile([128, 512], FP32, tag="pconv")
            nc.tensor.matmul(p[:, :128], lhsT=wh1_sb[:, 0:128], rhs=emb_T[:],
                             start=True, stop=True)
            nc.scalar.activation(h1a[:], p[:, :128], AF.Relu, bias=bh1_sb[:, 0:1])
            p2 = ps_conv.tile([128, 512], FP32, tag="pconv")
            nc.tensor.matmul(p2[:, :128], lhsT=wh1_sb[:, 128:256], rhs=emb_T[:],
                             start=True, stop=True)
            nc.scalar.activation(h1b[:], p2[:, :128], AF.Relu, bias=bh1_sb[:, 1:2])

            plog = ps_agg.tile([128, 128], FP32, tag="pagg")
            nc.tensor.matmul(plog[:NCLS, :], lhsT=wh2_sb[:, 0, :], rhs=h1a[:],
                             start=True, stop=False)
            nc.tensor.matmul(plog[:NCLS, :], lhsT=wh2_sb[:, 1, :], rhs=h1b[:],
                             start=False, stop=True)
            rawT = res.tile([128, 128], FP32)
            nc.vector.memset(rawT[:], 0.0)
            nc.scalar.activation(rawT[:NCLS, :], plog[:NCLS, :], AF.Identity,
                                 bias=bh2_sb[:NCLS, :])
            pt2 = ps_t.tile([128, 128], FP32, tag="pt")
            nc.tensor.transpose(pt2[:], rawT[:], id_sb[:])
            raw_nm = res.tile([128, NCLS], FP32)
            nc.scalar.activation(raw_nm[:], pt2[:, :NCLS], AF.Copy)

            # log_softmax over free axis (10)
            mx = res.tile([128, 1], FP32)
            nc.vector.tensor_reduce(mx[:], raw_nm[:], axis=AX.X, op=ALU.max)
            xs = res.tile([128, NCLS], FP32)
            nc.vector.tensor_tensor(out=xs[:], in0=raw_nm[:],
                                    in1=mx[:].to_broadcast([128, NCLS]),
                                    op=ALU.subtract)
            ex = res.tile([128, NCLS], FP32)
            nc.scalar.activation(ex[:], xs[:], AF.Exp)
            ssum = res.tile([128, 1], FP32)
            nc.vector.tensor_reduce(ssum[:], ex[:], axis=AX.X, op=ALU.add)
            ls = res.tile([128, 1], FP32)
            nc.scalar.activation(ls[:], ssum[:], AF.Ln)
            logout = res.tile([128, NCLS], FP32)
            nc.vector.tensor_tensor(out=logout[:], in0=xs[:],
                                    in1=ls[:].to_broadcast([128, NCLS]),
                                    op=ALU.subtract)
            nc.sync.dma_start(logits_o[:], logout[:])

            # loss / acc
            tmp10 = res.tile([128, NCLS], FP32)
            nc.vector.tensor_tensor(out=tmp10[:], in0=raw_nm[:], in1=lblm_sb[:],
                                    op=ALU.mult)
            pick_raw = res.tile([128, 1], FP32)
            nc.vector.tensor_reduce(pick_raw[:], tmp10[:], axis=AX.X, op=ALU.add)
            stack2 = res.tile([128, 2], FP32)
            nc.vector.tensor_tensor(out=stack2[:, 1:2], in0=pick_raw[:],
                                    in1=mx[:], op=ALU.is_equal)
            tmp10b = res.tile([128, NCLS], FP32)
            nc.vector.tensor_tensor(out=tmp10b[:], in0=logout[:], in1=lblm_sb[:],
                                    op=ALU.mult)
            nc.vector.tensor_reduce(stack2[:, 0:1], tmp10b[:], axis=AX.X,
                                    op=ALU.add)
            ps_s = ps_t.tile([128, 128], FP32, tag="pt")
            nc.tensor.matmul(ps_s[:2, :1], lhsT=stack2[:], rhs=ones_sb[:],
                             start=True, stop=True)
            fin = res.tile([128, 1], FP32)
            nc.vector.tensor_tensor(out=fin[:2, :], in0=ps_s[:2, :1],
                                    in1=msc_sb[:2, :], op=ALU.mult)
            nc.sync.dma_start(loss_o[:], fin[0:1, :])
            nc.sync.dma_start(acc_o[:], fin[1:2, :])

    nc.compile()
    return nc


_CACHE = {}


def _get_nc(T_lo, T_hi):
    import os
    variant = os.environ.get("KVAR", "full")
    max_lv = int(os.environ.get("KLV", str(MAX_LV)))
    key = (T_lo, T_hi, variant, max_lv)
    if key not in _CACHE:
        _CACHE[key] = build(T_lo, T_hi, variant, max_lv)
    return _CACHE[key]


def run(inputs, trace=False):
    pre = preprocess(inputs["node_feat"], inputs["edge_src"],
                     inputs["edge_dst"], inputs["graph_id"])
    T_lo, T_hi = pre["T_lo"], pre["T_hi"]
    nc = _get_nc(T_lo, T_hi)

    labels = np.asarray(inputs["labels"])
    lblmask = np.zeros((G, NCLS), np.float32)
    lblmask[np.arange(G), labels] = 1.0
    iota = np.tile(np.arange(128, dtype=np.float32), (128, 1))
    ident = np.eye(128, dtype=np.float32)
    mscale = np.zeros((128, 1), np.float32)
    mscale[0, 0] = -1.0 / G
    mscale[1, 0] = 1.0 / G
    ones = np.ones((128, 1), np.float32)
    b128 = lambda v: np.asarray(v, np.float32).reshape(128, 1)
    bh1 = np.asarray(inputs["b_h1"], np.float32).reshape(2, 128).T.copy()
    bh2 = np.zeros((128, 1), np.float32)
    bh2[:NCLS, 0] = np.asarray(inputs["b_h2"], np.float32)
    wh2T = np.asarray(inputs["w_h2"], np.float32).T  # [256, 10]
    wh2T2 = np.stack([wh2T[:128], wh2T[128:]], axis=1).copy()  # [128, 2, 10]

    shared = dict(
        lblmask=lblmask, iota=iota, ident=ident, mscale=mscale, ones=ones,
        wn2lT=np.ascontiguousarray(np.asarray(inputs["w_n2l"], np.float32).T),
        wconvT=np.ascontiguousarray(np.asarray(inputs["w_conv"], np.float32).T),
        woutT=np.ascontiguousarray(np.asarray(inputs["w_out"], np.float32).T),
        wh1T=np.ascontiguousarray(np.asarray(inputs["w_h1"], np.float32).T),
        wh2T2=wh2T2,
        bn2l=b128(inputs["b_n2l"]), bconv=b128(inputs["b_conv"]),
        bout=b128(inputs["b_out"]), bh1=bh1, bh2=bh2,
    )
    in_maps = []
    for c in range(NCORES):
        m = dict(shared)
        m["nf_T"] = pre["nf_T"][c]
        m["idx_lo"] = pre["idx_lo_w"][c]
        m["idx_hi"] = pre["idx_hi_w"][c]
        m["dst_lo"] = pre["dst_lo_w"][c]
        m["dst_hi"] = pre["dst_hi_w"][c]
        m["gid"] = pre["gid_nm"][c]
        in_maps.append(m)

    res = run_bass_kernel_spmd(nc, in_maps, core_ids=list(range(NCORES)),
                               trace=trace)
    r0 = res.results[0]
    logits = r0["logits"].astype(np.float32)
    loss = np.float32(r0["loss"].reshape(())[()])
    acc = np.float32(r0["acc"].reshape(())[()])
    return (logits, loss, acc), res


def kernel(**inputs):
    (logits, loss, acc), _ = run(inputs, trace=False)
    return logits, loss, acc
